# revision 27
# baseline (speedup 1.0000x reference)
"""EnVAE sampling kernel for 8x TRN2 NeuronCores — sorted-batch fused-selection design.

Math (per group g, batch element b):
  Xg = X[:, g::8]                                      # (b, 128)
  h  = relu(Xg @ W1[g] + b1[g])                        # (b, 128)
  out= h @ W2[g] + b2[g]; means=out[:, :64]; lv=out[:, 64:]
  z  = means[b, idx] + eps * exp(0.5 * lv[b, idx])

Key trick: each group g reads a DISJOINT column slice of X, so the host can
reorder each group's batch independently — sort by idx[g]. Then within any
128-column chunk of the sorted batch, at most ~3 distinct latents appear, and
mm2 + latent selection fuse into <=3 tiny matmuls per chunk:
  stationary = h-chunk [128 hid, 128 batch] (SBUF)
  moving     = the 2 columns of W2 for that run's latent (mean, logvar)
  out        = [128 batch, 2] cols of the per-group z psum tile
No onehot, no Hadamard, no on-device exp. Host finishes:
  z = zm + b2m[g, idx] + eps * exp(0.5*(zv + b2v[g, idx]))

Device mm1 runs fp8e4m3 in DoubleRow perf mode (2 contraction slots per
partition, X packed [64, 2, b]); W1 is pre-scaled by 16 to stay out of fp8
denormals and W2 pre-divided by 16 to compensate (relu(a*x) = a*relu(x)).
"""

import numpy as np
import ml_dtypes

import concourse.bass as bass
import concourse.bacc as bacc
import concourse.mybir as mybir
from concourse import tile
from concourse import bass_utils

OBS = 1024
LAT = 64
G = 8
GS = 128
HID = 128
BATCH = 65536
NCORES = 8
BPC = BATCH // NCORES        # 8192 batch rows per core
SC = 1024                    # batch rows per superchunk (relu granularity)
NSC = BPC // SC              # 8
CHUNK = 128                  # batch rows per mm2sel chunk (PE stationary width)
NCH = BPC // CHUNK           # 64 chunks per (group, core)
SEGS = 3                     # padded segments per chunk (fixed for SPMD)
ZC = NCH * SEGS * 2          # z cols per group = 384
W1SCALE = 16.0

FP8 = mybir.dt.float8e4
BF16 = mybir.dt.bfloat16
F32 = mybir.dt.float32
NP_FP8 = ml_dtypes.float8_e4m3
NP_BF16 = ml_dtypes.bfloat16

# group n takes columns n, n+8, ... (round-robin)
GROUP_IDX = np.stack([np.arange(n, OBS, G) for n in range(G)])  # (g, gs)


def build_program(num_devices: int = NCORES):
    """Per-core bass program (SPMD: identical across cores; per-core data
    differences live in xt / w2sel)."""
    nc = bacc.Bacc("TRN2", target_bir_lowering=False, debug=False,
                   num_devices=num_devices)

    # xt[g, p, sc, i, b] = Xg_sorted[sc*SC + b, p + 64*i]  (fp8)
    xt = nc.dram_tensor("xt", [G, 64, NSC, 2, SC], FP8, kind="ExternalInput").ap()
    # w1[p, g, i, m] = 16 * W1[g, p + 64*i, m]  (fp8)
    w1 = nc.dram_tensor("w1", [64, G, 2, HID], FP8, kind="ExternalInput").ap()
    # w2sel[k, g, ch, s, j] = W2[g, k, l(g,ch,s) + 64*j] / 16  (bf16)
    w2sel = nc.dram_tensor("w2sel", [HID, G, NCH, SEGS, 2], BF16,
                           kind="ExternalInput").ap()
    # b1s[k, g] = 16 * b1[g, k]
    b1 = nc.dram_tensor("b1", [HID, G], F32, kind="ExternalInput").ap()
    # zout[g][row, (ch*SEGS+s)*2 + j]: j=0 -> zm, j=1 -> zv  (bf16)
    zout = nc.dram_tensor("z", [G, CHUNK, ZC], BF16, kind="ExternalOutput").ap()

    # --- static engine load balancer for the vector ops -------------------
    # op cost model (ns) for [*, n]-col ops per engine; greedy least-loaded
    eng_time = {"act": 0.0, "dve": 0.0}

    def relu_cost(e, n):
        if e == "act":
            return n * 0.833 + 185.0
        return n * 1.042 + 125.0

    def pick_engine(n):
        e = min(eng_time, key=lambda k: eng_time[k] + relu_cost(k, n))
        eng_time[e] += relu_cost(e, n)
        return e

    from contextlib import ExitStack
    with tile.TileContext(nc) as tc, ExitStack() as st:
        cp = st.enter_context(tc.tile_pool(name="const", bufs=1))
        w1_sb = cp.tile([64, G, 2, HID], FP8, tag="w1")
        nc.sync.dma_start(w1_sb[:], w1)
        b1_sb = cp.tile([HID, G], F32, tag="b1")
        nc.sync.dma_start(b1_sb[:], b1)
        # pre-load the ACT function table while DMAs run (LoadActFuncSet is
        # ~1.3us and would otherwise serialize with the first relu)
        warm = cp.tile([1, 1], F32, tag="warm")
        nc.vector.memset(warm[:], 0.0)
        warm2 = cp.tile([1, 1], F32, tag="warm2")
        nc.scalar.activation(warm2[:], warm[:],
                             mybir.ActivationFunctionType.Relu,
                             bias=0.0, scale=1.0)

        xpool = st.enter_context(tc.tile_pool(name="xg", bufs=4))
        wspool = st.enter_context(tc.tile_pool(name="ws", bufs=2))
        hpool = st.enter_context(tc.tile_pool(name="hsb", bufs=10))
        # one zsb per group: a drain must NEVER wait on a zout DMA (those
        # queue behind xt transfers on the serialized DMA engines, and a
        # stalled drain blocks every later relu in its engine's in-order queue)
        zspool = st.enter_context(tc.tile_pool(name="zsb", bufs=8))
        hpsum = st.enter_context(tc.tile_pool(name="hp", bufs=3, space="PSUM"))
        zpsum = st.enter_context(tc.tile_pool(name="zt", bufs=2, space="PSUM"))

        relu_fns = {
            "act": lambda o, i, b: nc.scalar.activation(
                o, i, mybir.ActivationFunctionType.Relu, bias=b, scale=1.0),
            "dve": lambda o, i, b: nc.vector.tensor_scalar(
                o, i, b, 0.0, mybir.AluOpType.add, mybir.AluOpType.max),
        }
        copy_fns = {
            "act": nc.scalar.copy,
            "dve": nc.vector.tensor_copy,
        }


        # software-pipelined emission: PE sel-matmuls run one instance behind
        pending = []            # (g, sc, hsb, zt)
        gdone = []              # (g, zt) awaiting drain after last sel emitted

        def emit_sel(item):
            # one matmul per 128-batch chunk: moving = all SEGS*2 contiguous
            # W2 columns for that chunk (fewer PE instructions -> less queue
            # transit on the critical path)
            g, sc, hsb, zt = item
            w = SEGS * 2
            for cc in range(SC // CHUNK):
                ch = sc * (SC // CHUNK) + cc
                nc.tensor.matmul(
                    zt[:, ch * w:(ch + 1) * w],
                    hsb[:, CHUNK * cc:CHUNK * (cc + 1)],
                    wsel_tiles[g][:, g % 4, ch],
                    start=True, stop=True, skip_group_check=True)
            if sc == NSC - 1:
                gdone.append((g, zt))

        def emit_drain():
            g, zt = gdone.pop(0)
            e = pick_engine(ZC)
            zsb = zspool.tile([CHUNK, ZC], BF16, name=f"zsb{g}", tag="zsb")
            copy_fns[e](zsb[:], zt[:, :ZC])
            # ACT HWDGE: with one zsb per group nothing downstream waits on
            # these, and the Pool SWDGE path is pathologically slow under
            # the fake_nrt emulation used by the axon client.
            nc.scalar.dma_start(zout[g], zsb[:])

        wsel_tiles = {}
        for g in range(G):
            # prefetch xt for group g as two half-DMAs (amortizes the ~625ns
            # HWDGE fixed cost while keeping startup latency low)
            xg = xpool.tile([64, NSC, 2, SC], FP8, name=f"xg{g}", tag="xg")
            nparts = 4 if g == 0 else 2   # finer first DMA -> earlier start
            psc = NSC // nparts
            for h in range(nparts):
                nc.sync.dma_start(xg[:, h * psc:(h + 1) * psc],
                                  xt[g, :, h * psc:(h + 1) * psc])
            if g == 0:
                # both wsel DMAs upfront: emitting ws1 at g=4 would queue it
                # on SP behind slot-blocked xt DMAs, starving g>=4 sels
                for wh in (0, 1):
                    wsel = wspool.tile([HID, 4, NCH, SEGS, 2], BF16,
                                       name=f"ws{wh}", tag="wsel")
                    nc.sync.dma_start(wsel[:], w2sel[:, 4 * wh:4 * wh + 4])
                    for gg in range(4 * wh, 4 * wh + 4):
                        wsel_tiles[gg] = wsel
            zt = zpsum.tile([CHUNK, 512], F32, name=f"zt{g}", tag="zt")
            for sc in range(NSC):
                hp = hpsum.tile([HID, SC], F32, tag="hp")
                for half in range(SC // 512):
                    nc.tensor.matmul(
                        hp[:, 512 * half:512 * (half + 1)],
                        w1_sb[:, g],
                        xg[:, sc, :, 512 * half:512 * (half + 1)],
                        start=True, stop=True,
                        perf_mode=mybir.MatmulPerfMode.DoubleRow)
                hsb = hpool.tile([HID, SC], BF16, tag="hsb")
                e = pick_engine(SC)
                relu_fns[e](hsb[:], hp[:], b1_sb[:, g:g + 1])

                pending.append((g, sc, hsb, zt))
                # skew: keep sel-matmuls (which wait on relu i) from
                # head-of-line-blocking later mm1s in the in-order PE queue
                if len(pending) > 5:
                    emit_sel(pending.pop(0))
                # drain-skew: emit drains well after the group's last sels so
                # the drain never parks in ACT/DVE's in-order queue waiting
                if len(gdone) > 0 and (sc >= 3 or gdone[0][0] == g - 2):
                    emit_drain()
        while pending:
            emit_sel(pending.pop(0))
        while gdone:
            emit_drain()

    nc.compile()
    return nc


# ---------------------------------------------------------------- host side --

def _prep_host(X, eps, W1, b1, W2, b2, indices, ncores=NCORES):
    """Per-core input dicts + metadata for unscrambling."""
    W1p = np.ascontiguousarray(
        (W1 * W1SCALE).reshape(G, 2, 64, HID).transpose(2, 0, 1, 3)
    ).astype(NP_FP8)                                   # (64, G, 2, HID)
    b1s = np.ascontiguousarray((W1SCALE * b1).T).astype(np.float32)  # (HID, G)
    W2s = (W2 / W1SCALE).astype(np.float32)            # (G, HID, 128)

    in_maps = []
    metas = []
    for core in range(ncores):
        lo = core * BPC
        xt = np.empty((G, 64, NSC, 2, SC), NP_FP8)
        w2sel = np.empty((HID, G, NCH, SEGS, 2), NP_BF16)
        meta = []
        for g in range(G):
            idxg = indices[g, lo:lo + BPC]
            order = np.argsort(idxg, kind="stable")    # sorted batch positions
            slat = idxg[order]                         # (BPC,) sorted latents
            Xg = X[lo + order][:, GROUP_IDX[g]].astype(NP_FP8)  # (BPC, 128)
            # pack [p, sc, i, b]: col k = p + 64*i
            xt[g] = (Xg.reshape(NSC, SC, 2, 64)
                     .transpose(3, 0, 2, 1))           # (64, NSC, 2, SC)
            # segments: distinct latents per 128-chunk, padded to SEGS
            lat_ch = slat.reshape(NCH, CHUNK)
            seg_lat = np.zeros((NCH, SEGS), np.int64)
            seg_of_pos = np.empty(BPC, np.int64)
            for ch in range(NCH):
                uniq, inv = np.unique(lat_ch[ch], return_inverse=True)
                ns = len(uniq)
                assert ns <= SEGS, f"chunk needs {ns} segments > SEGS={SEGS}"
                seg_lat[ch, :ns] = uniq
                seg_of_pos[ch * CHUNK:(ch + 1) * CHUNK] = inv
            # w2sel[k, ch, s, j] = W2s[g][k, seg_lat[ch,s] + 64*j]
            cols = (seg_lat[None, :, :, None] +
                    64 * np.arange(2)[None, None, None, :])  # (1, NCH, SEGS, 2)
            w2sel[:, g] = W2s[g][:, cols[0]].astype(NP_BF16)
            meta.append((order, slat, seg_of_pos))
        in_maps.append({"xt": xt, "w1": W1p, "w2sel": w2sel, "b1": b1s})
        metas.append(meta)
    return in_maps, metas


def _finish_host(zdev, meta, eps_c, b2):
    """zdev: (G, CHUNK, ZC) f32; returns z (G, BPC) in original batch order."""
    z = np.empty((G, BPC), np.float32)
    pos = np.arange(BPC)
    rows = pos % CHUNK
    ch = pos // CHUNK
    for g in range(G):
        order, slat, seg_of_pos = meta[g]
        col = (ch * SEGS + seg_of_pos) * 2
        zm = zdev[g][rows, col]
        zv = zdev[g][rows, col + 1]
        zs = (zm + b2[g, slat] +
              eps_c[g, order] * np.exp(0.5 * (zv + b2[g, LAT + slat])))
        z[g, order] = zs
    return z


_NC_CACHE = {}


def kernel(X, eps, W1, b1, W2, b2, indices):
    if "nc" not in _NC_CACHE:
        _NC_CACHE["nc"] = build_program(NCORES)
    nc = _NC_CACHE["nc"]
    in_maps, metas = _prep_host(X, eps, W1, b1, W2, b2, indices)
    res = bass_utils.run_bass_kernel_spmd(nc, in_maps,
                                          core_ids=list(range(NCORES)))
    z = np.zeros((G, BATCH), np.float32)
    for core in range(NCORES):
        lo = core * BPC
        zdev = np.asarray(res.results[core]["z"]).astype(np.float32)
        z[:, lo:lo + BPC] = _finish_host(zdev, metas[core],
                                         eps[:, lo:lo + BPC], b2)
    return z.astype(np.float32)


# revision 29
# speedup vs baseline: 1.1100x; 1.1100x over previous
"""EnVAE sampling kernel for 8x TRN2 NeuronCores — sorted-batch fused-selection design.

Math (per group g, batch element b):
  Xg = X[:, g::8]                                      # (b, 128)
  h  = relu(Xg @ W1[g] + b1[g])                        # (b, 128)
  out= h @ W2[g] + b2[g]; means=out[:, :64]; lv=out[:, 64:]
  z  = means[b, idx] + eps * exp(0.5 * lv[b, idx])

Key trick: each group g reads a DISJOINT column slice of X, so the host can
reorder each group's batch independently — sort by idx[g]. Then within any
128-column chunk of the sorted batch, at most ~3 distinct latents appear, and
mm2 + latent selection fuse into <=3 tiny matmuls per chunk:
  stationary = h-chunk [128 hid, 128 batch] (SBUF)
  moving     = the 2 columns of W2 for that run's latent (mean, logvar)
  out        = [128 batch, 2] cols of the per-group z psum tile
No onehot, no Hadamard, no on-device exp. Host finishes:
  z = zm + b2m[g, idx] + eps * exp(0.5*(zv + b2v[g, idx]))

Device mm1 runs fp8e4m3 in DoubleRow perf mode (2 contraction slots per
partition, X packed [64, 2, b]); W1 is pre-scaled by 16 to stay out of fp8
denormals and W2 pre-divided by 16 to compensate (relu(a*x) = a*relu(x)).
"""

import numpy as np
import ml_dtypes

import concourse.bass as bass
import concourse.bacc as bacc
import concourse.mybir as mybir
from concourse import tile
from concourse import bass_utils

OBS = 1024
LAT = 64
G = 8
GS = 128
HID = 128
BATCH = 65536
NCORES = 8
BPC = BATCH // NCORES        # 8192 batch rows per core
SC = 1024                    # batch rows per superchunk (relu granularity)
NSC = BPC // SC              # 8
CHUNK = 128                  # batch rows per mm2sel chunk (PE stationary width)
NCH = BPC // CHUNK           # 64 chunks per (group, core)
SEGS = 3                     # padded segments per chunk (fixed for SPMD)
ZC = NCH * SEGS * 2          # z cols per group = 384
W1SCALE = 16.0

FP8 = mybir.dt.float8e4
BF16 = mybir.dt.bfloat16
F32 = mybir.dt.float32
NP_FP8 = ml_dtypes.float8_e4m3
NP_BF16 = ml_dtypes.bfloat16

# group n takes columns n, n+8, ... (round-robin)
GROUP_IDX = np.stack([np.arange(n, OBS, G) for n in range(G)])  # (g, gs)


def build_program(num_devices: int = NCORES):
    """Per-core bass program (SPMD: identical across cores; per-core data
    differences live in xt / w2sel)."""
    nc = bacc.Bacc("TRN2", target_bir_lowering=False, debug=False,
                   num_devices=num_devices)

    # xt[g, p, sc, i, b] = Xg_sorted[sc*SC + b, p + 64*i]  (fp8)
    xt = nc.dram_tensor("xt", [G, 64, NSC, 2, SC], FP8, kind="ExternalInput").ap()
    # w1[p, g, i, m] = 16 * W1[g, p + 64*i, m]  (fp8)
    w1 = nc.dram_tensor("w1", [64, G, 2, HID], FP8, kind="ExternalInput").ap()
    # w2sel[k, g, ch, s, j] = W2[g, k, l(g,ch,s) + 64*j] / 16  (bf16)
    w2sel = nc.dram_tensor("w2sel", [HID, G, NCH, SEGS, 2], BF16,
                           kind="ExternalInput").ap()
    # b1s[k, g] = 16 * b1[g, k]
    b1 = nc.dram_tensor("b1", [HID, G], F32, kind="ExternalInput").ap()
    # zout[g][row, (ch*SEGS+s)*2 + j]: j=0 -> zm, j=1 -> zv  (bf16)
    zout = nc.dram_tensor("z", [G, CHUNK, ZC], BF16, kind="ExternalOutput").ap()

    # --- static engine load balancer for the vector ops -------------------
    # op cost model (ns) for [*, n]-col ops per engine; greedy least-loaded
    eng_time = {"act": 0.0, "dve": 0.0}

    def relu_cost(e, n):
        if e == "act":
            return n * 0.833 + 185.0
        return n * 1.042 + 125.0

    def pick_engine(n):
        e = min(eng_time, key=lambda k: eng_time[k] + relu_cost(k, n))
        eng_time[e] += relu_cost(e, n)
        return e

    from contextlib import ExitStack
    with tile.TileContext(nc) as tc, ExitStack() as st:
        cp = st.enter_context(tc.tile_pool(name="const", bufs=1))
        w1_sb = cp.tile([64, G, 2, HID], FP8, tag="w1")
        nc.sync.dma_start(w1_sb[:], w1)
        b1_sb = cp.tile([HID, G], F32, tag="b1")
        nc.sync.dma_start(b1_sb[:], b1)
        # pre-load the ACT function table while DMAs run (LoadActFuncSet is
        # ~1.3us and would otherwise serialize with the first relu)
        warm = cp.tile([1, 1], F32, tag="warm")
        nc.vector.memset(warm[:], 0.0)
        warm2 = cp.tile([1, 1], F32, tag="warm2")
        nc.scalar.activation(warm2[:], warm[:],
                             mybir.ActivationFunctionType.Relu,
                             bias=0.0, scale=1.0)

        xpool = st.enter_context(tc.tile_pool(name="xg", bufs=4))
        wspool = st.enter_context(tc.tile_pool(name="ws", bufs=2))
        hpool = st.enter_context(tc.tile_pool(name="hsb", bufs=10))
        # one zsb per group: a drain must NEVER wait on a zout DMA (those
        # queue behind xt transfers on the serialized DMA engines, and a
        # stalled drain blocks every later relu in its engine's in-order queue)
        zspool = st.enter_context(tc.tile_pool(name="zsb", bufs=8))
        hpsum = st.enter_context(tc.tile_pool(name="hp", bufs=3, space="PSUM"))
        zpsum = st.enter_context(tc.tile_pool(name="zt", bufs=2, space="PSUM"))

        relu_fns = {
            "act": lambda o, i, b: nc.scalar.activation(
                o, i, mybir.ActivationFunctionType.Relu, bias=b, scale=1.0),
            "dve": lambda o, i, b: nc.vector.tensor_scalar(
                o, i, b, 0.0, mybir.AluOpType.add, mybir.AluOpType.max),
        }
        copy_fns = {
            "act": nc.scalar.copy,
            "dve": nc.vector.tensor_copy,
        }


        # software-pipelined emission: PE sel-matmuls run one instance behind
        pending = []            # (g, sc, hsb, zt)
        gdone = []              # (g, zt) awaiting drain after last sel emitted

        def emit_sel(item):
            # one matmul per 128-batch chunk: moving = all SEGS*2 contiguous
            # W2 columns for that chunk (fewer PE instructions -> less queue
            # transit on the critical path)
            g, sc, hsb, zt = item
            w = SEGS * 2
            for cc in range(SC // CHUNK):
                ch = sc * (SC // CHUNK) + cc
                nc.tensor.matmul(
                    zt[:, ch * w:(ch + 1) * w],
                    hsb[:, CHUNK * cc:CHUNK * (cc + 1)],
                    wsel_tiles[g][:, g % 4, ch],
                    start=True, stop=True, skip_group_check=True)
            if sc == NSC - 1:
                gdone.append((g, zt))

        def emit_drain():
            g, zt = gdone.pop(0)
            e = pick_engine(ZC)
            zsb = zspool.tile([CHUNK, ZC], BF16, name=f"zsb{g}", tag="zsb")
            copy_fns[e](zsb[:], zt[:, :ZC])
            # ACT HWDGE: with one zsb per group nothing downstream waits on
            # these, and the Pool SWDGE path is pathologically slow under
            # the fake_nrt emulation used by the axon client.
            nc.scalar.dma_start(zout[g], zsb[:])

        wsel_tiles = {}
        for g in range(G):
            # prefetch xt for group g as two half-DMAs (amortizes the ~625ns
            # HWDGE fixed cost while keeping startup latency low)
            xg = xpool.tile([64, NSC, 2, SC], FP8, name=f"xg{g}", tag="xg")
            nparts = 4 if g == 0 else 2   # finer first DMA -> earlier start
            psc = NSC // nparts
            for h in range(nparts):
                nc.sync.dma_start(xg[:, h * psc:(h + 1) * psc],
                                  xt[g, :, h * psc:(h + 1) * psc])
            if g == 0:
                # both wsel DMAs upfront: emitting ws1 at g=4 would queue it
                # on SP behind slot-blocked xt DMAs, starving g>=4 sels
                for wh in (0, 1):
                    wsel = wspool.tile([HID, 4, NCH, SEGS, 2], BF16,
                                       name=f"ws{wh}", tag="wsel")
                    nc.sync.dma_start(wsel[:], w2sel[:, 4 * wh:4 * wh + 4])
                    for gg in range(4 * wh, 4 * wh + 4):
                        wsel_tiles[gg] = wsel
            zt = zpsum.tile([CHUNK, 512], F32, name=f"zt{g}", tag="zt")
            for sc in range(NSC):
                hp = hpsum.tile([HID, SC], F32, tag="hp")
                for half in range(SC // 512):
                    nc.tensor.matmul(
                        hp[:, 512 * half:512 * (half + 1)],
                        w1_sb[:, g],
                        xg[:, sc, :, 512 * half:512 * (half + 1)],
                        start=True, stop=True,
                        perf_mode=mybir.MatmulPerfMode.DoubleRow)
                hsb = hpool.tile([HID, SC], BF16, tag="hsb")
                e = pick_engine(SC)
                relu_fns[e](hsb[:], hp[:], b1_sb[:, g:g + 1])

                pending.append((g, sc, hsb, zt))
                # skew: keep sel-matmuls (which wait on relu i) from
                # head-of-line-blocking later mm1s in the in-order PE queue
                if len(pending) > 5:
                    emit_sel(pending.pop(0))
                # drain-skew: emit drains well after the group's last sels so
                # the drain never parks in ACT/DVE's in-order queue waiting
                if len(gdone) > 0 and (sc >= 3 or gdone[0][0] == g - 2):
                    emit_drain()
        while pending:
            emit_sel(pending.pop(0))
        while gdone:
            emit_drain()

    nc.compile()
    return nc


# ---------------------------------------------------------------- host side --

def _prep_host(X, eps, W1, b1, W2, b2, indices, ncores=NCORES):
    """Per-core input dicts + metadata for unscrambling."""
    W1p = np.ascontiguousarray(
        (W1 * W1SCALE).reshape(G, 2, 64, HID).transpose(2, 0, 1, 3)
    ).astype(NP_FP8)                                   # (64, G, 2, HID)
    b1s = np.ascontiguousarray((W1SCALE * b1).T).astype(np.float32)  # (HID, G)
    W2s = (W2 / W1SCALE).astype(np.float32)            # (G, HID, 128)

    in_maps = []
    metas = []
    for core in range(ncores):
        lo = core * BPC
        xt = np.empty((G, 64, NSC, 2, SC), NP_FP8)
        w2sel = np.empty((HID, G, NCH, SEGS, 2), NP_BF16)
        meta = []
        for g in range(G):
            idxg = indices[g, lo:lo + BPC]
            order = np.argsort(idxg, kind="stable")    # sorted batch positions
            slat = idxg[order]                         # (BPC,) sorted latents
            Xg = X[lo + order][:, GROUP_IDX[g]].astype(NP_FP8)  # (BPC, 128)
            # pack [p, sc, i, b]: col k = p + 64*i
            xt[g] = (Xg.reshape(NSC, SC, 2, 64)
                     .transpose(3, 0, 2, 1))           # (64, NSC, 2, SC)
            # segments: distinct latents per 128-chunk, padded to SEGS
            lat_ch = slat.reshape(NCH, CHUNK)
            seg_lat = np.zeros((NCH, SEGS), np.int64)
            seg_of_pos = np.empty(BPC, np.int64)
            for ch in range(NCH):
                uniq, inv = np.unique(lat_ch[ch], return_inverse=True)
                ns = len(uniq)
                assert ns <= SEGS, f"chunk needs {ns} segments > SEGS={SEGS}"
                seg_lat[ch, :ns] = uniq
                seg_of_pos[ch * CHUNK:(ch + 1) * CHUNK] = inv
            # w2sel[k, ch, s, j] = W2s[g][k, seg_lat[ch,s] + 64*j]
            cols = (seg_lat[None, :, :, None] +
                    64 * np.arange(2)[None, None, None, :])  # (1, NCH, SEGS, 2)
            w2sel[:, g] = W2s[g][:, cols[0]].astype(NP_BF16)
            meta.append((order, slat, seg_of_pos))
        in_maps.append({"xt": xt, "w1": W1p, "w2sel": w2sel, "b1": b1s})
        metas.append(meta)
    return in_maps, metas


def _finish_host(zdev, meta, eps_c, b2):
    """zdev: (G, CHUNK, ZC) f32; returns z (G, BPC) in original batch order."""
    z = np.empty((G, BPC), np.float32)
    pos = np.arange(BPC)
    rows = pos % CHUNK
    ch = pos // CHUNK
    for g in range(G):
        order, slat, seg_of_pos = meta[g]
        col = (ch * SEGS + seg_of_pos) * 2
        zm = zdev[g][rows, col]
        zv = zdev[g][rows, col + 1]
        zs = (zm + b2[g, slat] +
              eps_c[g, order] * np.exp(0.5 * (zv + b2[g, LAT + slat])))
        z[g, order] = zs
    return z


# ---------------------------------------------------------- raw-bass build --
# Manual-semaphore version (no TileContext): exact point-to-point sync,
# strict ACT/DVE relu alternation. ~11% faster than the tile build.

SKEW = 9          # sels for instance i emitted after mm1(i+SKEW)
HPB = 3           # hp psum buffers (2 banks each)
HSBB = 14         # hsb sbuf buffers (even -> same-engine WAW, no cross wait)
XGB = 4           # xg group buffers


def build_program_raw(num_devices: int = NCORES):
    nc = bacc.Bacc("TRN2", target_bir_lowering=False, debug=False,
                   num_devices=num_devices)

    xt = nc.dram_tensor("xt", [G, 64, NSC, 2, SC], FP8, kind="ExternalInput").ap()
    w1 = nc.dram_tensor("w1", [64, G, 2, HID], FP8, kind="ExternalInput").ap()
    w2sel = nc.dram_tensor("w2sel", [HID, G, NCH, SEGS, 2], BF16,
                           kind="ExternalInput").ap()
    b1 = nc.dram_tensor("b1", [HID, G], F32, kind="ExternalInput").ap()
    zout = nc.dram_tensor("z", [G, CHUNK, ZC], BF16, kind="ExternalOutput").ap()

    # ---- SBUF / PSUM -----------------------------------------------------
    w1_sb = nc.alloc_sbuf_tensor("w1s", [64, G, 2, HID], FP8).ap()
    b1_sb = nc.alloc_sbuf_tensor("b1s", [HID, G], F32).ap()
    ws_sb = nc.alloc_sbuf_tensor("wss", [HID, G, NCH, SEGS, 2], BF16).ap()
    xg = [nc.alloc_sbuf_tensor(f"xg{k}", [64, NSC, 2, SC], FP8).ap()
          for k in range(XGB)]
    hsb = [nc.alloc_sbuf_tensor(f"hsb{k}", [HID, SC], BF16).ap()
           for k in range(HSBB)]
    zsb = [nc.alloc_sbuf_tensor(f"zsb{k}", [CHUNK, ZC], BF16).ap()
           for k in range(G)]
    warm = nc.alloc_sbuf_tensor("warm", [1, 1], F32).ap()
    hp = [nc.alloc_psum_tensor(f"hp{k}", [HID, SC], F32).ap() for k in range(HPB)]
    zt = [nc.alloc_psum_tensor(f"zt{k}", [CHUNK, 512], F32).ap() for k in range(2)]

    # ---- semaphores ------------------------------------------------------
    s_w = nc.alloc_semaphore("s_w")       # const/wsel DMAs done (1:w1 2:b1 3:ws)
    s_x = nc.alloc_semaphore("s_x")       # xt DMA pieces done
    s_mm1 = nc.alloc_semaphore("s_mm1")   # mm1 instances done (PE)
    s_sel = nc.alloc_semaphore("s_sel")   # sel instances done (PE)
    s_ra = nc.alloc_semaphore("s_ra")     # ACT relus done
    s_rd = nc.alloc_semaphore("s_rd")     # DVE relus done
    s_da = nc.alloc_semaphore("s_da")     # ACT drains done
    s_dd = nc.alloc_semaphore("s_dd")     # DVE drains done
    s_zo = nc.alloc_semaphore("s_zo")     # zout DMAs done (walrus needs an update)

    def relu_sem(i):
        return (s_ra, i // 2 + 1) if i % 2 == 0 else (s_rd, i // 2 + 1)

    # ---- SP: DMA stream --------------------------------------------------
    nc.sync.dma_start(w1_sb, w1).then_inc(s_w, 16)
    nc.sync.dma_start(b1_sb, b1).then_inc(s_w, 16)
    xpieces = 0
    # group 0 in quarters for early start, then the wsel load, then halves
    q = NSC // 4
    for h in range(4):
        nc.sync.dma_start(xg[0][:, q * h:q * h + q], xt[0, :, q * h:q * h + q]
                          ).then_inc(s_x, 16)
        xpieces += 1
    nc.sync.dma_start(ws_sb, w2sel).then_inc(s_w, 16)
    xneed = {}   # (g, sc) -> required s_x value
    for sc in range(NSC):
        xneed[(0, sc)] = sc // q + 1
    for g in range(1, G):
        hh = NSC // 2
        for h in range(2):
            ins = nc.sync.dma_start(
                xg[g % XGB][:, hh * h:hh * h + hh], xt[g, :, hh * h:hh * h + hh])
            if g >= XGB:
                # slot recycle: all mm1s of group g-XGB consumed xg[g%XGB]
                ins.wait_op(s_mm1, NSC * (g - XGB + 1), "sem-ge")
            ins.then_inc(s_x, 16)
            xpieces += 1
        for sc in range(NSC):
            xneed[(g, sc)] = 4 + 2 * (g - 1) + sc // hh + 1

    # ---- PE stream -------------------------------------------------------
    nc.tensor.wait_ge(s_w, 16)    # w1 loaded, before any mm1

    def emit_mm1(i):
        g, sc = divmod(i, NSC)
        hpi = hp[i % HPB]
        if i >= HPB:
            sem, val = relu_sem(i - HPB)
            nc.tensor.wait_ge(sem, val)
        nhalf = SC // 512
        for half in range(nhalf):
            ins = nc.tensor.matmul(
                hpi[:, 512 * half:512 * half + 512],
                w1_sb[:, g],
                xg[g % XGB][:, sc, :, 512 * half:512 * half + 512],
                start=True, stop=True,
                perf_mode=mybir.MatmulPerfMode.DoubleRow)
            if half == 0:
                ins.wait_op(s_x, 16 * xneed[(g, sc)], "sem-ge")
            if half == nhalf - 1:
                ins.then_inc(s_mm1, 1)

    def emit_sels(i):
        g, sc = divmod(i, NSC)
        w = SEGS * 2
        zti = zt[g % 2]
        if i == 0:
            nc.tensor.wait_ge(s_w, 48)   # wsel loaded
        if sc == 0 and g >= 2:
            # zt slot recycle: drain(g-2) must have copied it out
            dsem = s_da if (g - 2) % 2 == 0 else s_dd
            nc.tensor.wait_ge(dsem, (g - 2) // 2 + 1)
        for cc in range(SC // CHUNK):
            ch = sc * (SC // CHUNK) + cc
            ins = nc.tensor.matmul(
                zti[:, ch * w:(ch + 1) * w],
                hsb[i % HSBB][:, CHUNK * cc:CHUNK * (cc + 1)],
                ws_sb[:, g, ch],
                start=True, stop=True, skip_group_check=True)
            if cc == 0:
                sem, val = relu_sem(i)
                ins.wait_op(sem, val, "sem-ge")
            if cc == SC // CHUNK - 1:
                ins.then_inc(s_sel, 1)

    # ---- ACT / DVE streams ----------------------------------------------
    nc.vector.memset(warm, 0.0)
    nc.scalar.activation(warm, warm, mybir.ActivationFunctionType.Relu,
                         bias=0.0, scale=1.0)

    def emit_relu(i):
        g, sc = divmod(i, NSC)
        eng_act = (i % 2 == 0)
        o, inp = hsb[i % HSBB], hp[i % HPB]
        bias = b1_sb[:, g:g + 1]
        eng = nc.scalar if eng_act else nc.vector
        if i < 2:
            eng.wait_ge(s_w, 32)
        if i >= HSBB:
            # hsb slot readers (sels of i-HSBB) must be done; writer is the
            # same engine (HSBB even) so WAW is implied by in-order exec
            eng.wait_ge(s_sel, i - HSBB + 1)
        if eng_act:
            ins = nc.scalar.activation(o, inp,
                                       mybir.ActivationFunctionType.Relu,
                                       bias=bias, scale=1.0)
        else:
            ins = nc.vector.tensor_scalar(o, inp, bias, 0.0,
                                          mybir.AluOpType.add,
                                          mybir.AluOpType.max)
        ins.wait_op(s_mm1, i + 1, "sem-ge")
        ins.then_inc(s_ra if eng_act else s_rd, 1)

    def emit_drain(g):
        eng_act = (g % 2 == 0)
        e = nc.scalar if eng_act else nc.vector
        src = zt[g % 2][:, :ZC]
        if eng_act:
            ins = e.copy(zsb[g], src)
        else:
            ins = e.tensor_copy(zsb[g], src)
        ins.wait_op(s_sel, NSC * (g + 1), "sem-ge")
        ins.then_inc(s_da if eng_act else s_dd, 1)

    def emit_zout(g):
        ins = nc.scalar.dma_start(zout[g], zsb[g])
        dsem = s_da if g % 2 == 0 else s_dd
        ins.wait_op(dsem, g // 2 + 1, "sem-ge")
        ins.then_inc(s_zo, 16)

    # ---- interleaved emission -------------------------------------------
    drains_due = []
    for i in range(G * NSC):
        emit_mm1(i)
        if i >= SKEW:
            emit_sels(i - SKEW)
            g_done, sc_done = divmod(i - SKEW, NSC)
            if sc_done == NSC - 1:
                drains_due.append(g_done)
        emit_relu(i)
        # emit drain/zout a couple instances after the group's sels
        if drains_due and i % NSC >= 2:
            g_d = drains_due.pop(0)
            emit_drain(g_d)
            emit_zout(g_d)
    for i in range(G * NSC - SKEW, G * NSC):
        emit_sels(i)
        if i % NSC == NSC - 1:
            drains_due.append(i // NSC)
    while drains_due:
        g_d = drains_due.pop(0)
        emit_drain(g_d)
        emit_zout(g_d)

    nc.compile()
    return nc




build_program_tile = build_program
build_program = build_program_raw

_NC_CACHE = {}


def kernel(X, eps, W1, b1, W2, b2, indices):
    if "nc" not in _NC_CACHE:
        _NC_CACHE["nc"] = build_program(NCORES)
    nc = _NC_CACHE["nc"]
    in_maps, metas = _prep_host(X, eps, W1, b1, W2, b2, indices)
    res = bass_utils.run_bass_kernel_spmd(nc, in_maps,
                                          core_ids=list(range(NCORES)))
    z = np.zeros((G, BATCH), np.float32)
    for core in range(NCORES):
        lo = core * BPC
        zdev = np.asarray(res.results[core]["z"]).astype(np.float32)
        z[:, lo:lo + BPC] = _finish_host(zdev, metas[core],
                                         eps[:, lo:lo + BPC], b2)
    return z.astype(np.float32)


# revision 32
# speedup vs baseline: 1.1903x; 1.0724x over previous
"""EnVAE sampling kernel for 8x TRN2 NeuronCores — sorted-batch fused-selection design.

Math (per group g, batch element b):
  Xg = X[:, g::8]                                      # (b, 128)
  h  = relu(Xg @ W1[g] + b1[g])                        # (b, 128)
  out= h @ W2[g] + b2[g]; means=out[:, :64]; lv=out[:, 64:]
  z  = means[b, idx] + eps * exp(0.5 * lv[b, idx])

Key trick: each group g reads a DISJOINT column slice of X, so the host can
reorder each group's batch independently — sort by idx[g]. Then within any
128-column chunk of the sorted batch, at most ~3 distinct latents appear, and
mm2 + latent selection fuse into <=3 tiny matmuls per chunk:
  stationary = h-chunk [128 hid, 128 batch] (SBUF)
  moving     = the 2 columns of W2 for that run's latent (mean, logvar)
  out        = [128 batch, 2] cols of the per-group z psum tile
No onehot, no Hadamard, no on-device exp. Host finishes:
  z = zm + b2m[g, idx] + eps * exp(0.5*(zv + b2v[g, idx]))

Device mm1 runs fp8e4m3 in DoubleRow perf mode (2 contraction slots per
partition, X packed [64, 2, b]); W1 is pre-scaled by 16 to stay out of fp8
denormals and W2 pre-divided by 16 to compensate (relu(a*x) = a*relu(x)).
"""

import numpy as np
import ml_dtypes

import concourse.bass as bass
import concourse.bacc as bacc
import concourse.mybir as mybir
from concourse import tile
from concourse import bass_utils

OBS = 1024
LAT = 64
G = 8
GS = 128
HID = 128
BATCH = 65536
NCORES = 8
BPC = BATCH // NCORES        # 8192 batch rows per core
SC = 1024                    # batch rows per superchunk (relu granularity)
NSC = BPC // SC              # 8
CHUNK = 128                  # batch rows per mm2sel chunk (PE stationary width)
NCH = BPC // CHUNK           # 64 chunks per (group, core)
SEGS = 3                     # padded segments per chunk (fixed for SPMD)
ZC = NCH * SEGS * 2          # z cols per group = 384
W1SCALE = 16.0

FP8 = mybir.dt.float8e4
BF16 = mybir.dt.bfloat16
F32 = mybir.dt.float32
NP_FP8 = ml_dtypes.float8_e4m3
NP_BF16 = ml_dtypes.bfloat16

# group n takes columns n, n+8, ... (round-robin)
GROUP_IDX = np.stack([np.arange(n, OBS, G) for n in range(G)])  # (g, gs)


def build_program(num_devices: int = NCORES):
    """Per-core bass program (SPMD: identical across cores; per-core data
    differences live in xt / w2sel)."""
    nc = bacc.Bacc("TRN2", target_bir_lowering=False, debug=False,
                   num_devices=num_devices)

    # xt[g, p, sc, i, b] = Xg_sorted[sc*SC + b, p + 64*i]  (fp8)
    xt = nc.dram_tensor("xt", [G, 64, NSC, 2, SC], FP8, kind="ExternalInput").ap()
    # w1[p, g, i, m] = 16 * W1[g, p + 64*i, m]  (fp8)
    w1 = nc.dram_tensor("w1", [64, G, 2, HID], FP8, kind="ExternalInput").ap()
    # w2sel[k, g, ch, s, j] = W2[g, k, l(g,ch,s) + 64*j] / 16  (bf16)
    w2sel = nc.dram_tensor("w2sel", [HID, G, NCH, SEGS, 2], BF16,
                           kind="ExternalInput").ap()
    # b1s[k, g] = 16 * b1[g, k]
    b1 = nc.dram_tensor("b1", [HID, G], F32, kind="ExternalInput").ap()
    # zout[g][row, (ch*SEGS+s)*2 + j]: j=0 -> zm, j=1 -> zv  (bf16)
    zout = nc.dram_tensor("z", [G, CHUNK, ZC], BF16, kind="ExternalOutput").ap()

    # --- static engine load balancer for the vector ops -------------------
    # op cost model (ns) for [*, n]-col ops per engine; greedy least-loaded
    eng_time = {"act": 0.0, "dve": 0.0}

    def relu_cost(e, n):
        if e == "act":
            return n * 0.833 + 185.0
        return n * 1.042 + 125.0

    def pick_engine(n):
        e = min(eng_time, key=lambda k: eng_time[k] + relu_cost(k, n))
        eng_time[e] += relu_cost(e, n)
        return e

    from contextlib import ExitStack
    with tile.TileContext(nc) as tc, ExitStack() as st:
        cp = st.enter_context(tc.tile_pool(name="const", bufs=1))
        w1_sb = cp.tile([64, G, 2, HID], FP8, tag="w1")
        nc.sync.dma_start(w1_sb[:], w1)
        b1_sb = cp.tile([HID, G], F32, tag="b1")
        nc.sync.dma_start(b1_sb[:], b1)
        # pre-load the ACT function table while DMAs run (LoadActFuncSet is
        # ~1.3us and would otherwise serialize with the first relu)
        warm = cp.tile([1, 1], F32, tag="warm")
        nc.vector.memset(warm[:], 0.0)
        warm2 = cp.tile([1, 1], F32, tag="warm2")
        nc.scalar.activation(warm2[:], warm[:],
                             mybir.ActivationFunctionType.Relu,
                             bias=0.0, scale=1.0)

        xpool = st.enter_context(tc.tile_pool(name="xg", bufs=4))
        wspool = st.enter_context(tc.tile_pool(name="ws", bufs=2))
        hpool = st.enter_context(tc.tile_pool(name="hsb", bufs=10))
        # one zsb per group: a drain must NEVER wait on a zout DMA (those
        # queue behind xt transfers on the serialized DMA engines, and a
        # stalled drain blocks every later relu in its engine's in-order queue)
        zspool = st.enter_context(tc.tile_pool(name="zsb", bufs=8))
        hpsum = st.enter_context(tc.tile_pool(name="hp", bufs=3, space="PSUM"))
        zpsum = st.enter_context(tc.tile_pool(name="zt", bufs=2, space="PSUM"))

        relu_fns = {
            "act": lambda o, i, b: nc.scalar.activation(
                o, i, mybir.ActivationFunctionType.Relu, bias=b, scale=1.0),
            "dve": lambda o, i, b: nc.vector.tensor_scalar(
                o, i, b, 0.0, mybir.AluOpType.add, mybir.AluOpType.max),
        }
        copy_fns = {
            "act": nc.scalar.copy,
            "dve": nc.vector.tensor_copy,
        }


        # software-pipelined emission: PE sel-matmuls run one instance behind
        pending = []            # (g, sc, hsb, zt)
        gdone = []              # (g, zt) awaiting drain after last sel emitted

        def emit_sel(item):
            # one matmul per 128-batch chunk: moving = all SEGS*2 contiguous
            # W2 columns for that chunk (fewer PE instructions -> less queue
            # transit on the critical path)
            g, sc, hsb, zt = item
            w = SEGS * 2
            for cc in range(SC // CHUNK):
                ch = sc * (SC // CHUNK) + cc
                nc.tensor.matmul(
                    zt[:, ch * w:(ch + 1) * w],
                    hsb[:, CHUNK * cc:CHUNK * (cc + 1)],
                    wsel_tiles[g][:, g % 4, ch],
                    start=True, stop=True, skip_group_check=True)
            if sc == NSC - 1:
                gdone.append((g, zt))

        def emit_drain():
            g, zt = gdone.pop(0)
            e = pick_engine(ZC)
            zsb = zspool.tile([CHUNK, ZC], BF16, name=f"zsb{g}", tag="zsb")
            copy_fns[e](zsb[:], zt[:, :ZC])
            # ACT HWDGE: with one zsb per group nothing downstream waits on
            # these, and the Pool SWDGE path is pathologically slow under
            # the fake_nrt emulation used by the axon client.
            nc.scalar.dma_start(zout[g], zsb[:])

        wsel_tiles = {}
        for g in range(G):
            # prefetch xt for group g as two half-DMAs (amortizes the ~625ns
            # HWDGE fixed cost while keeping startup latency low)
            xg = xpool.tile([64, NSC, 2, SC], FP8, name=f"xg{g}", tag="xg")
            nparts = 4 if g == 0 else 2   # finer first DMA -> earlier start
            psc = NSC // nparts
            for h in range(nparts):
                nc.sync.dma_start(xg[:, h * psc:(h + 1) * psc],
                                  xt[g, :, h * psc:(h + 1) * psc])
            if g == 0:
                # both wsel DMAs upfront: emitting ws1 at g=4 would queue it
                # on SP behind slot-blocked xt DMAs, starving g>=4 sels
                for wh in (0, 1):
                    wsel = wspool.tile([HID, 4, NCH, SEGS, 2], BF16,
                                       name=f"ws{wh}", tag="wsel")
                    nc.sync.dma_start(wsel[:], w2sel[:, 4 * wh:4 * wh + 4])
                    for gg in range(4 * wh, 4 * wh + 4):
                        wsel_tiles[gg] = wsel
            zt = zpsum.tile([CHUNK, 512], F32, name=f"zt{g}", tag="zt")
            for sc in range(NSC):
                hp = hpsum.tile([HID, SC], F32, tag="hp")
                for half in range(SC // 512):
                    nc.tensor.matmul(
                        hp[:, 512 * half:512 * (half + 1)],
                        w1_sb[:, g],
                        xg[:, sc, :, 512 * half:512 * (half + 1)],
                        start=True, stop=True,
                        perf_mode=mybir.MatmulPerfMode.DoubleRow)
                hsb = hpool.tile([HID, SC], BF16, tag="hsb")
                e = pick_engine(SC)
                relu_fns[e](hsb[:], hp[:], b1_sb[:, g:g + 1])

                pending.append((g, sc, hsb, zt))
                # skew: keep sel-matmuls (which wait on relu i) from
                # head-of-line-blocking later mm1s in the in-order PE queue
                if len(pending) > 5:
                    emit_sel(pending.pop(0))
                # drain-skew: emit drains well after the group's last sels so
                # the drain never parks in ACT/DVE's in-order queue waiting
                if len(gdone) > 0 and (sc >= 3 or gdone[0][0] == g - 2):
                    emit_drain()
        while pending:
            emit_sel(pending.pop(0))
        while gdone:
            emit_drain()

    nc.compile()
    return nc


# ---------------------------------------------------------------- host side --

def _prep_host(X, eps, W1, b1, W2, b2, indices, ncores=NCORES):
    """Per-core input dicts + metadata for unscrambling."""
    W1p = np.ascontiguousarray(
        (W1 * W1SCALE).reshape(G, 2, 64, HID).transpose(2, 0, 1, 3)
    ).astype(NP_FP8)                                   # (64, G, 2, HID)
    b1s = np.ascontiguousarray((W1SCALE * b1).T).astype(np.float32)  # (HID, G)
    W2s = (W2 / W1SCALE).astype(np.float32)            # (G, HID, 128)

    in_maps = []
    metas = []
    for core in range(ncores):
        lo = core * BPC
        xt = np.empty((G, 64, NSC, 2, SC), NP_FP8)
        w2sel = np.empty((HID, G, NCH, SEGS, 2), NP_BF16)
        meta = []
        for g in range(G):
            idxg = indices[g, lo:lo + BPC]
            order = np.argsort(idxg, kind="stable")    # sorted batch positions
            slat = idxg[order]                         # (BPC,) sorted latents
            Xg = X[lo + order][:, GROUP_IDX[g]].astype(NP_FP8)  # (BPC, 128)
            # pack [p, sc, i, b]: col k = p + 64*i
            xt[g] = (Xg.reshape(NSC, SC, 2, 64)
                     .transpose(3, 0, 2, 1))           # (64, NSC, 2, SC)
            # segments: distinct latents per 128-chunk, padded to SEGS
            lat_ch = slat.reshape(NCH, CHUNK)
            seg_lat = np.zeros((NCH, SEGS), np.int64)
            seg_of_pos = np.empty(BPC, np.int64)
            for ch in range(NCH):
                uniq, inv = np.unique(lat_ch[ch], return_inverse=True)
                ns = len(uniq)
                assert ns <= SEGS, f"chunk needs {ns} segments > SEGS={SEGS}"
                seg_lat[ch, :ns] = uniq
                seg_of_pos[ch * CHUNK:(ch + 1) * CHUNK] = inv
            # w2sel[k, ch, s, j] = W2s[g][k, seg_lat[ch,s] + 64*j]
            cols = (seg_lat[None, :, :, None] +
                    64 * np.arange(2)[None, None, None, :])  # (1, NCH, SEGS, 2)
            w2sel[:, g] = W2s[g][:, cols[0]].astype(NP_BF16)
            meta.append((order, slat, seg_of_pos))
        in_maps.append({"xt": xt, "w1": W1p, "w2sel": w2sel, "b1": b1s})
        metas.append(meta)
    return in_maps, metas


def _finish_host(zdev, meta, eps_c, b2):
    """zdev: (G, CHUNK, ZC) f32; returns z (G, BPC) in original batch order."""
    z = np.empty((G, BPC), np.float32)
    pos = np.arange(BPC)
    rows = pos % CHUNK
    ch = pos // CHUNK
    for g in range(G):
        order, slat, seg_of_pos = meta[g]
        col = (ch * SEGS + seg_of_pos) * 2
        zm = zdev[g][rows, col]
        zv = zdev[g][rows, col + 1]
        zs = (zm + b2[g, slat] +
              eps_c[g, order] * np.exp(0.5 * (zv + b2[g, LAT + slat])))
        z[g, order] = zs
    return z


# ---------------------------------------------------------- raw-bass build --
# Manual-semaphore version (no TileContext): exact point-to-point sync,
# strict ACT/DVE relu alternation. ~11% faster than the tile build.

SKEW = 9          # sels for instance i emitted after mm1(i+SKEW)
HPB = 3           # hp psum buffers (2 banks each)
HSBB = 14         # hsb sbuf buffers (even -> same-engine WAW, no cross wait)
XGB = 4           # xg group buffers


def build_program_raw(num_devices: int = NCORES):
    nc = bacc.Bacc("TRN2", target_bir_lowering=False, debug=False,
                   num_devices=num_devices)

    xt = nc.dram_tensor("xt", [G, 64, NSC, 2, SC], FP8, kind="ExternalInput").ap()
    w1 = nc.dram_tensor("w1", [64, G, 2, HID], FP8, kind="ExternalInput").ap()
    w2sel = nc.dram_tensor("w2sel", [HID, G, NCH, SEGS, 2], BF16,
                           kind="ExternalInput").ap()
    b1 = nc.dram_tensor("b1", [HID, G], F32, kind="ExternalInput").ap()
    zout = nc.dram_tensor("z", [G, CHUNK, ZC], BF16, kind="ExternalOutput").ap()

    # ---- SBUF / PSUM -----------------------------------------------------
    w1_sb = nc.alloc_sbuf_tensor("w1s", [64, G, 2, HID], FP8).ap()
    b1_sb = nc.alloc_sbuf_tensor("b1s", [HID, G], F32).ap()
    ws_sb = nc.alloc_sbuf_tensor("wss", [HID, G, NCH, SEGS, 2], BF16).ap()
    xg = [nc.alloc_sbuf_tensor(f"xg{k}", [64, NSC, 2, SC], FP8).ap()
          for k in range(XGB)]
    hsb = [nc.alloc_sbuf_tensor(f"hsb{k}", [HID, SC], BF16).ap()
           for k in range(HSBB)]
    zsb = [nc.alloc_sbuf_tensor(f"zsb{k}", [CHUNK, ZC], BF16).ap()
           for k in range(G)]
    warm = nc.alloc_sbuf_tensor("warm", [1, 1], F32).ap()
    hp = [nc.alloc_psum_tensor(f"hp{k}", [HID, SC], F32).ap() for k in range(HPB)]
    zt = [nc.alloc_psum_tensor(f"zt{k}", [CHUNK, 512], F32).ap() for k in range(2)]

    # ---- semaphores ------------------------------------------------------
    s_w = nc.alloc_semaphore("s_w")       # const/wsel DMAs done (1:w1 2:b1 3:ws)
    s_x = nc.alloc_semaphore("s_x")       # xt DMA pieces done
    s_mm1 = nc.alloc_semaphore("s_mm1")   # mm1 instances done (PE)
    s_sel = nc.alloc_semaphore("s_sel")   # sel instances done (PE)
    s_ra = nc.alloc_semaphore("s_ra")     # ACT relus done
    s_rd = nc.alloc_semaphore("s_rd")     # DVE relus done
    s_da = nc.alloc_semaphore("s_da")     # ACT drains done
    s_dd = nc.alloc_semaphore("s_dd")     # DVE drains done
    s_zo = nc.alloc_semaphore("s_zo")     # zout DMAs done (walrus needs an update)
    s_d7 = nc.alloc_semaphore("s_d7")     # last group's half-drains done

    def relu_sem(i):
        return (s_ra, i // 2 + 1) if i % 2 == 0 else (s_rd, i // 2 + 1)

    # ---- SP: DMA stream --------------------------------------------------
    nc.sync.dma_start(w1_sb, w1).then_inc(s_w, 16)
    xpieces = 0
    # group 0 front-loaded: single-sc first pieces for the earliest start
    g0_pieces = [1, 1, 2, 4]   # sc widths, sum = NSC
    off = 0
    xneed = {}   # (g, sc) -> required s_x value
    for pi, wdt in enumerate(g0_pieces):
        nc.sync.dma_start(xg[0][:, off:off + wdt], xt[0, :, off:off + wdt]
                          ).then_inc(s_x, 16)
        for sc in range(off, off + wdt):
            xneed[(0, sc)] = pi + 1
        off += wdt
        xpieces += 1
        if pi == 0:
            nc.sync.dma_start(b1_sb, b1).then_inc(s_w, 16)
    nc.sync.dma_start(ws_sb[:, 0:4], w2sel[:, 0:4]).then_inc(s_w, 16)
    for g in range(1, G):
        hh = NSC // 2
        for h in range(2):
            ins = nc.sync.dma_start(
                xg[g % XGB][:, hh * h:hh * h + hh], xt[g, :, hh * h:hh * h + hh])
            if g == 1 and h == 0:
                nc.sync.dma_start(ws_sb[:, 4:8], w2sel[:, 4:8]).then_inc(s_w, 16)
            if g >= XGB:
                # slot recycle: all mm1s of group g-XGB consumed xg[g%XGB]
                ins.wait_op(s_mm1, NSC * (g - XGB + 1), "sem-ge")
            ins.then_inc(s_x, 16)
            xpieces += 1
        for sc in range(NSC):
            xneed[(g, sc)] = 4 + 2 * (g - 1) + sc // hh + 1

    # ---- PE stream -------------------------------------------------------
    nc.tensor.wait_ge(s_w, 16)    # w1 loaded, before any mm1

    def emit_mm1(i):
        g, sc = divmod(i, NSC)
        hpi = hp[i % HPB]
        if i >= HPB:
            sem, val = relu_sem(i - HPB)
            nc.tensor.wait_ge(sem, val)
        nhalf = SC // 512
        for half in range(nhalf):
            ins = nc.tensor.matmul(
                hpi[:, 512 * half:512 * half + 512],
                w1_sb[:, g],
                xg[g % XGB][:, sc, :, 512 * half:512 * half + 512],
                start=True, stop=True,
                perf_mode=mybir.MatmulPerfMode.DoubleRow)
            if half == 0:
                ins.wait_op(s_x, 16 * xneed[(g, sc)], "sem-ge")
            if half == nhalf - 1:
                ins.then_inc(s_mm1, 1)

    def emit_sels(i):
        g, sc = divmod(i, NSC)
        w = SEGS * 2
        zti = zt[g % 2]
        if i == 0:
            nc.tensor.wait_ge(s_w, 48)   # first wsel half loaded
        if i == 4 * NSC:
            nc.tensor.wait_ge(s_w, 64)   # second wsel half loaded
        if sc == 0 and g >= 2:
            # zt slot recycle: drain(g-2) must have copied it out
            dsem = s_da if (g - 2) % 2 == 0 else s_dd
            nc.tensor.wait_ge(dsem, (g - 2) // 2 + 1)
        for cc in range(SC // CHUNK):
            ch = sc * (SC // CHUNK) + cc
            ins = nc.tensor.matmul(
                zti[:, ch * w:(ch + 1) * w],
                hsb[i % HSBB][:, CHUNK * cc:CHUNK * (cc + 1)],
                ws_sb[:, g, ch],
                start=True, stop=True, skip_group_check=True)
            if cc == 0:
                sem, val = relu_sem(i)
                ins.wait_op(sem, val, "sem-ge")
            if cc == SC // CHUNK - 1:
                ins.then_inc(s_sel, 1)

    # ---- ACT / DVE streams ----------------------------------------------
    nc.vector.memset(warm, 0.0)
    nc.scalar.activation(warm, warm, mybir.ActivationFunctionType.Relu,
                         bias=0.0, scale=1.0)

    def emit_relu(i):
        g, sc = divmod(i, NSC)
        eng_act = (i % 2 == 0)
        o, inp = hsb[i % HSBB], hp[i % HPB]
        bias = b1_sb[:, g:g + 1]
        eng = nc.scalar if eng_act else nc.vector
        if i < 2:
            eng.wait_ge(s_w, 32)
        if i >= HSBB:
            # hsb slot readers (sels of i-HSBB) must be done; writer is the
            # same engine (HSBB even) so WAW is implied by in-order exec
            eng.wait_ge(s_sel, i - HSBB + 1)
        if eng_act:
            ins = nc.scalar.activation(o, inp,
                                       mybir.ActivationFunctionType.Relu,
                                       bias=bias, scale=1.0)
        else:
            ins = nc.vector.tensor_scalar(o, inp, bias, 0.0,
                                          mybir.AluOpType.add,
                                          mybir.AluOpType.max)
        ins.wait_op(s_mm1, i + 1, "sem-ge")
        ins.then_inc(s_ra if eng_act else s_rd, 1)

    def emit_drain7(half):
        # last group's drain in halves so the tail only pays half a drain
        lo, hi = (0, ZC // 2) if half == 0 else (ZC // 2, ZC)
        ins = nc.scalar.copy(zsb[G - 1][:, lo:hi], zt[(G - 1) % 2][:, lo:hi])
        ins.wait_op(s_sel, NSC * (G - 1) + NSC // 2 * (half + 1), "sem-ge")
        ins.then_inc(s_d7, 1)

    def emit_zout7(half):
        lo, hi = (0, ZC // 2) if half == 0 else (ZC // 2, ZC)
        ins = nc.scalar.dma_start(zout[G - 1][:, lo:hi], zsb[G - 1][:, lo:hi])
        ins.wait_op(s_d7, half + 1, "sem-ge")
        ins.then_inc(s_zo, 16)

    def emit_drain(g):
        eng_act = (g % 2 == 0)
        e = nc.scalar if eng_act else nc.vector
        src = zt[g % 2][:, :ZC]
        if eng_act:
            ins = e.copy(zsb[g], src)
        else:
            ins = e.tensor_copy(zsb[g], src)
        ins.wait_op(s_sel, NSC * (g + 1), "sem-ge")
        ins.then_inc(s_da if eng_act else s_dd, 1)

    def emit_zout(g):
        ins = nc.scalar.dma_start(zout[g], zsb[g])
        dsem = s_da if g % 2 == 0 else s_dd
        ins.wait_op(dsem, g // 2 + 1, "sem-ge")
        ins.then_inc(s_zo, 16)

    # ---- interleaved emission -------------------------------------------
    drains_due = []
    zouts_due = []   # (g, emit_at_instance): zout waits must be satisfied at
                     # decode or they hold ACT.SEQ and stall later relus
    for i in range(G * NSC):
        emit_mm1(i)
        if i >= SKEW:
            emit_sels(i - SKEW)
            g_done, sc_done = divmod(i - SKEW, NSC)
            if sc_done == NSC - 1:
                drains_due.append(g_done)
        emit_relu(i)
        if drains_due and i % NSC >= 2:
            g_d = drains_due.pop(0)
            emit_drain(g_d)
            zouts_due.append((g_d, i + 3))
        while zouts_due and zouts_due[0][1] <= i:
            emit_zout(zouts_due.pop(0)[0])
    for i in range(G * NSC - SKEW, G * NSC):
        emit_sels(i)
        g_s, sc_s = divmod(i, NSC)
        if i % NSC == NSC - 1:
            drains_due.append(g_s)
    while zouts_due:
        emit_zout(zouts_due.pop(0)[0])
    while drains_due:
        g_d = drains_due.pop(0)
        emit_drain(g_d)
        emit_zout(g_d)


    nc.compile()
    return nc




build_program_tile = build_program
build_program = build_program_raw

_NC_CACHE = {}


def kernel(X, eps, W1, b1, W2, b2, indices):
    if "nc" not in _NC_CACHE:
        _NC_CACHE["nc"] = build_program(NCORES)
    nc = _NC_CACHE["nc"]
    in_maps, metas = _prep_host(X, eps, W1, b1, W2, b2, indices)
    res = bass_utils.run_bass_kernel_spmd(nc, in_maps,
                                          core_ids=list(range(NCORES)))
    z = np.zeros((G, BATCH), np.float32)
    for core in range(NCORES):
        lo = core * BPC
        zdev = np.asarray(res.results[core]["z"]).astype(np.float32)
        z[:, lo:lo + BPC] = _finish_host(zdev, metas[core],
                                         eps[:, lo:lo + BPC], b2)
    return z.astype(np.float32)
